# revision 17
# baseline (speedup 1.0000x reference)
# GCNConv (dense adjacency, symmetric normalization) on 8 trn2 NeuronCores.
#
#   out = D^{-1/2} A D^{-1/2} (x @ W) + bias,   deg = A.sum(axis=1)
#
# Strategy (row-shard, 1D graph partition):
#   - core c owns output rows [1024c, 1024(c+1)). Its 32MB shard of A is
#     laid out host-side in the exact SBUF target layout ("packed"):
#     pack[p, ic, b, i] = A[rows_c[ic*512+i], b*128+p], i.e. the shard
#     transposed (contraction index j = b*128+p on the partition axis) and
#     i-chunked. Every load DMA is then a plain 2D slice with 16KB
#     contiguous per partition - full HBM streaming rate.
#   - The shard is DMA-cast fp32->bf16 (SWDGE) and stays SBUF-resident
#     (16MB): one HBM pass over A (the memory roofline).
#   - deg (row sums of A) = ones^T @ adjT on the tensor engine, accumulated
#     over the 64 j-blocks; a tiny AllGather distributes deg. The local i
#     axis is chunked so the first AllGather and the SpMM work it unlocks
#     overlap the second half of the load. A dummy warm-up AllGather at
#     t=0 absorbs the ~50us first-collective/rank-skew cost.
#   - dinv = 1/sqrt(deg) via ACT sqrt + DVE reciprocal + one Newton step.
#   - h = x @ W from host-transposed xT (replicated); H' = dinv*h in bf16 is
#     the stationary operand of the SpMM:
#        outT[d, i] += sum_j H'[j, d] * adjT[j, i]
#     accumulated in PSUM over j-blocks, transposed back, scaled by local
#     dinv rows, bias added, DMA'd out.
#
# Engine-queue separation (each engine's instruction stream is in-order, so
# collective-gated waits must never sit ahead of bulk work):
#   gpsimd: warm-up AG, collective doorbells
#   sync:   bulk fp32 loads (HWDGE), AllGather bounce DMAs, final output DMA
#   scalar: x/W/bias loads, h PSUM->SBUF copies, sqrt, outT copies
#   vector: deg copies, rsqrt/Newton, H' scaling, output scale+bias

import numpy as np

N = 8192
D = 128
NCORES = 8
P = 128


def _params(n, ncores):
    nb = n // P  # j-blocks (64)
    rpc = n // ncores  # rows per core (1024)
    nhalf = min(512, rpc)  # out slice width (PSUM bank limit)
    nslice = rpc // nhalf
    NCH = nslice  # i-chunks == out slices
    ich = rpc // NCH  # i-chunk width (512)
    return nb, rpc, nhalf, nslice, NCH, ich


def _build(n=N, d=D, ncores=NCORES):
    from contextlib import ExitStack

    import concourse.bacc as bacc
    import concourse.masks as masks
    import concourse.mybir as mybir
    import concourse.tile as tile

    f32 = mybir.dt.float32
    bf16 = mybir.dt.bfloat16
    mult = mybir.AluOpType.mult
    add = mybir.AluOpType.add

    nb, rpc, nhalf, nslice, NCH, ich = _params(n, ncores)
    lb = rpc // P  # local row tiles (8)
    cw = nb * ich  # AT columns per chunk
    bpd = min(8, nb)  # j-blocks per load DMA (2MB fp32)
    lbc = lb // NCH  # local row tiles per chunk

    def chunk_of_block(b):
        return (b % lb) // lbc

    def col_in_chunk(b):
        return lbc * (b // lb) + (b % lb) - chunk_of_block(b) * lbc

    nc = bacc.Bacc("TRN2", target_bir_lowering=False, debug=False, num_devices=ncores)

    adjp = nc.dram_tensor("adjp", [P, NCH * cw], f32, kind="ExternalInput")
    xT = nc.dram_tensor("xT", [d, n], f32, kind="ExternalInput")
    w = nc.dram_tensor("w", [d, d], f32, kind="ExternalInput")
    bias = nc.dram_tensor("bias", [d], f32, kind="ExternalInput")
    out = nc.dram_tensor("out", [rpc, d], f32, kind="ExternalOutput")

    with tile.TileContext(nc) as tc, ExitStack() as ctx:
        singles = ctx.enter_context(tc.tile_pool(name="singles", bufs=1))
        dram = ctx.enter_context(tc.tile_pool(name="dram", bufs=1, space="DRAM"))
        atp = ctx.enter_context(tc.tile_pool(name="atp", bufs=1))
        stp = ctx.enter_context(tc.tile_pool(name="stp", bufs=2))
        xcp = ctx.enter_context(tc.tile_pool(name="xcp", bufs=2))
        psdeg = ctx.enter_context(tc.tile_pool(name="psdeg", bufs=1, space="PSUM"))
        psh = ctx.enter_context(tc.tile_pool(name="psh", bufs=2, space="PSUM"))
        psout = ctx.enter_context(tc.tile_pool(name="psout", bufs=1, space="PSUM"))
        psmisc = ctx.enter_context(tc.tile_pool(name="psmisc", bufs=2, space="PSUM"))

        # ---- warm-up AllGather (first-collective cost hides under the load)
        wa_in = dram.tile([P], f32, name="wa_in")
        wa_out = dram.tile([ncores * P], f32, name="wa_out", addr_space="Shared")
        wa_sb = singles.tile([1, P], f32)
        nc.gpsimd.memset(wa_sb[:], 0.0)
        nc.gpsimd.dma_start(wa_in[:], wa_sb[:1, :])
        nc.gpsimd.collective_compute(
            "AllGather",
            mybir.AluOpType.bypass,
            replica_groups=[list(range(ncores))],
            ins=[wa_in.opt()],
            outs=[wa_out.opt()],
        )

        # ---- constants ----
        ident = singles.tile([P, P], f32)
        masks.make_identity(nc, ident[:])
        ones_bf = singles.tile([P, 1], bf16)
        nc.gpsimd.memset(ones_bf[:], 1.0)
        ones_row = singles.tile([1, P], f32)
        nc.gpsimd.memset(ones_row[:], 1.0)
        w_sb = singles.tile([d, d], f32)
        nc.scalar.dma_start(w_sb[:], w[:, :])
        bias_row = singles.tile([1, d], f32)
        nc.scalar.dma_start(bias_row[:], bias[:])
        bias_mat = singles.tile([P, d], f32)
        bm_ps = psmisc.tile([P, d], f32, tag="misc")
        nc.tensor.matmul(bm_ps[:], ones_row[:], bias_row[:])
        nc.vector.tensor_copy(bias_mat[:], bm_ps[:])

        # ---- big SBUF residents ----
        AT = atp.tile([P, NCH * cw], bf16)  # packed adjT, bf16
        Hb = singles.tile([P, nb * d], bf16)  # h then H' (in place), [j, (b d)]

        # ---- h = x @ W, interleaved into chunk-0's load (see below) ----
        xch = min(1024, n)

        def h_group(hg):
            c0 = hg * xch
            if c0 >= n:
                return
            xc = xcp.tile([d, xch], f32)
            nc.scalar.dma_start(xc[:], xT[:, c0 : c0 + xch])
            for bb in range(xch // P):
                b = c0 // P + bb
                h_ps = psh.tile([P, d], f32)
                nc.tensor.matmul(h_ps[:], xc[:, bb * P : (bb + 1) * P], w_sb[:])
                nc.scalar.copy(Hb[:, b * d : (b + 1) * d], h_ps[:])

        deg_ps = [psdeg.tile([1, ich], f32, name=f"deg_ps{ic}") for ic in range(NCH)]
        out_ps = [psout.tile([P, nhalf], f32, name=f"out_ps{s}") for s in range(nslice)]
        deg_sb = singles.tile([1, rpc], f32)
        dinv_ch = [singles.tile([P, nb // NCH], f32, name=f"dinv_ch{ic}") for ic in range(NCH)]
        ag_outs = []

        def rsqrt_newton(dst, deg_psum, width, tag):
            dgc = singles.tile([P, width], f32, name=f"dgc_{tag}")
            nc.vector.tensor_copy(dgc[:], deg_psum[:])
            sq = singles.tile([P, width], f32, name=f"sq_{tag}")
            nc.scalar.sqrt(sq[:], deg_psum[:])
            r0 = singles.tile([P, width], f32, name=f"r0_{tag}")
            nc.vector.reciprocal(r0[:], sq[:])
            t0 = singles.tile([P, width], f32, name=f"t0_{tag}")
            nc.vector.tensor_mul(t0[:], r0[:], r0[:])
            nc.vector.tensor_mul(t0[:], t0[:], dgc[:])
            nc.vector.tensor_scalar(t0[:], t0[:], -0.5, 1.5, mult, add)
            nc.vector.tensor_mul(dst[:], t0[:], r0[:])

        def ag_chain(ic):
            nc.vector.tensor_copy(deg_sb[:, ic * ich : (ic + 1) * ich], deg_ps[ic][:])
            ag_in = dram.tile([ich], f32, name=f"ag_in{ic}")
            ag_out = dram.tile([ncores * ich], f32, name=f"ag_out{ic}", addr_space="Shared")
            nc.gpsimd.dma_start(ag_in[:], deg_sb[:1, ic * ich : (ic + 1) * ich])
            nc.gpsimd.collective_compute(
                "AllGather",
                mybir.AluOpType.bypass,
                replica_groups=[list(range(ncores))],
                ins=[ag_in.opt()],
                outs=[ag_out.opt()],
            )
            ag_outs.append(ag_out)

        def dinv_chain(ic):
            nbc = nb // NCH
            degc = singles.tile([nbc, P], f32, name=f"degc{ic}")
            nc.gpsimd.dma_start(degc[:], ag_outs[ic][:])
            dgt_ps = psmisc.tile([P, nbc], f32, tag="misc")
            nc.tensor.transpose(dgt_ps[:], degc[:], ident[:nbc, :nbc])
            rsqrt_newton(dinv_ch[ic], dgt_ps, nbc, f"g{ic}")

        def spmm_block(b):
            col = col_in_chunk(b)
            nc.vector.tensor_scalar(
                Hb[:, b * d : (b + 1) * d],
                Hb[:, b * d : (b + 1) * d],
                dinv_ch[chunk_of_block(b)][:, col : col + 1],
                None,
                mult,
            )
            for s in range(nslice):
                nc.tensor.matmul(
                    out_ps[s][:],
                    Hb[:, b * d : (b + 1) * d],
                    AT[:, s * cw + b * ich : s * cw + (b + 1) * ich],
                    start=(b == 0),
                    stop=(b == nb - 1),
                    skip_group_check=True,
                )

        # ---- stream the shard in (SWDGE fp32->bf16 cast), deg, AGs; earlier
        # chunks' SpMM interleaves into the last chunk's load ----
        ngr = nb // bpd  # load groups per chunk
        SHIFT = 3
        for ic in range(NCH):
            for g, b0 in enumerate(range(0, nb, bpd)):
                lo = ic * cw + b0 * ich
                hi = ic * cw + (b0 + bpd) * ich
                stage = stp.tile([P, bpd * ich], f32)
                nc.sync.dma_start(stage[:], adjp[:, lo:hi])
                nc.vector.tensor_scalar(AT[:, lo:hi], stage[:], 1.0, None, mult)
                for b in range(b0, b0 + bpd):
                    nc.tensor.matmul(
                        deg_ps[ic][:],
                        ones_bf[:],
                        AT[:, ic * cw + b * ich : ic * cw + (b + 1) * ich],
                        start=(b == 0),
                        stop=(b == nb - 1),
                    )
                if ic == 0:
                    h_group(g)
                if ic == NCH - 1 and NCH > 1:
                    if g == min(2, ngr - 1):
                        dinv_chain(0)
                    if g >= SHIFT:
                        for b in range((g - SHIFT) * bpd, (g - SHIFT + 1) * bpd):
                            if chunk_of_block(b) < NCH - 1:
                                spmm_block(b)
            if ic == 0:
                for hg in range(ngr, (n + xch - 1) // xch):
                    h_group(hg)
            ag_chain(ic)
            if ic == NCH - 1 and NCH > 1:
                for g in range(max(0, ngr - SHIFT), ngr):
                    for b in range(g * bpd, (g + 1) * bpd):
                        if chunk_of_block(b) < NCH - 1:
                            spmm_block(b)

        # ---- last chunk's (or single-chunk) dinv + SpMM ----
        dinv_chain(NCH - 1)
        for b in range(nb):
            if chunk_of_block(b) == NCH - 1:
                spmm_block(b)

        # local dinv for this core's output rows, [p, r] layout
        dloc_ps = psmisc.tile([P, lb], f32, tag="misc")
        for r in range(lb):
            nc.tensor.transpose(
                dloc_ps[:, r : r + 1], deg_sb[:1, r * P : (r + 1) * P], ident[:1, :1]
            )
        dinvl = singles.tile([P, lb], f32)
        rsqrt_newton(dinvl, dloc_ps, lb, "l")

        # ---- finalize: transpose back, scale by dinv rows, add bias ----
        outT_sb = singles.tile([P, rpc], f32)
        for s in range(nslice):
            nc.scalar.copy(outT_sb[:, s * nhalf : (s + 1) * nhalf], out_ps[s][:])
        out_sb = singles.tile([P, lb * d], f32)
        for r in range(lb):
            ob_ps = psmisc.tile([P, d], f32, tag="misc")
            nc.tensor.transpose(ob_ps[:], outT_sb[:, r * P : (r + 1) * P], ident[:])
            nc.vector.tensor_scalar(
                out_sb[:, r * d : (r + 1) * d], ob_ps[:], dinvl[:, r : r + 1], None, mult
            )
            nc.vector.tensor_add(
                out_sb[:, r * d : (r + 1) * d], out_sb[:, r * d : (r + 1) * d], bias_mat[:]
            )
        nc.sync.dma_start(
            out.ap().rearrange("(r p) d -> p r d", p=P),
            out_sb[:].rearrange("p (r d) -> p r d", d=d),
        )

    nc.compile()
    return nc


_NC_CACHE = {}


def _get_nc(n=N, d=D, ncores=NCORES):
    key = (n, d, ncores)
    if key not in _NC_CACHE:
        _NC_CACHE[key] = _build(n, d, ncores)
    return _NC_CACHE[key]


def _pack_shard(adj, c, n, ncores):
    # pack[p, ic, b, i] = adj[c*rpc + ic*ich + i, b*P + p], flattened 2D.
    nb, rpc, nhalf, nslice, NCH, ich = _params(n, ncores)
    shard = adj[c * rpc : (c + 1) * rpc, :]  # [rpc, n]
    t = shard.T.reshape(nb, P, NCH, ich)  # [b, p, ic, i]
    return np.ascontiguousarray(t.transpose(1, 2, 0, 3).reshape(P, NCH * nb * ich))


def run(x, adj, weight, bias, n=N, d=D, ncores=NCORES, trace=False):
    from concourse import bass_utils

    x = np.ascontiguousarray(np.asarray(x, dtype=np.float32))
    adj = np.ascontiguousarray(np.asarray(adj, dtype=np.float32))
    weight = np.ascontiguousarray(np.asarray(weight, dtype=np.float32))
    bias = np.ascontiguousarray(np.asarray(bias, dtype=np.float32))

    xTa = np.ascontiguousarray(x.T)
    in_maps = []
    for c in range(ncores):
        in_maps.append(
            {"adjp": _pack_shard(adj, c, n, ncores), "xT": xTa, "w": weight, "bias": bias}
        )

    nc = _get_nc(n, d, ncores)
    res = bass_utils.run_bass_kernel_spmd(
        nc, in_maps, core_ids=list(range(ncores)), trace=trace
    )
    out = np.concatenate([r["out"] for r in res.results], axis=0)
    return out, res


def kernel(x, adj, weight, bias):
    out, _ = run(x, adj, weight, bias)
    return out


# revision 19
# speedup vs baseline: 1.0333x; 1.0333x over previous
# GCNConv (dense adjacency, symmetric normalization) on 8 trn2 NeuronCores.
#
#   out = D^{-1/2} A D^{-1/2} (x @ W) + bias,   deg = A.sum(axis=1)
#
# Strategy (row-shard, 1D graph partition):
#   - core c owns output rows [1024c, 1024(c+1)). Its 32MB shard of A is
#     laid out host-side in the exact SBUF target layout ("packed"):
#     pack[p, ic, b, i] = A[rows_c[ic*512+i], b*128+p]  - the shard
#     transposed (contraction index j = b*128+p on the partition axis) and
#     i-chunked, so every load DMA is a plain 2D slice with 16KB contiguous
#     per partition (full HBM streaming rate).
#   - The shard streams in fp32 (HWDGE), is cast to bf16 on the vector
#     engine, and stays SBUF-resident (16MB): one HBM pass over A.
#   - deg (row sums of A) = ones^T @ adjT on the tensor engine; a tiny
#     AllGather distributes deg. The local i axis is split in two chunks so
#     AG#0 and the SpMM it unlocks overlap the second half of the load; a
#     dummy warm-up AllGather at t=0 absorbs the ~50us first-collective
#     cost so the real AGs run at ~8us.
#   - dinv = 1/sqrt(deg) via ACT sqrt + DVE reciprocal + one Newton step.
#   - h = x @ W from host-transposed xT (replicated); H' = dinv*h in bf16 is
#     the stationary operand of the SpMM:
#        outT[d, i] += sum_j H'[j, d] * adjT[j, i]
#     accumulated in PSUM over j-blocks, transposed back, scaled by local
#     dinv rows, bias added, DMA'd out.
#
# Engine queues are in-order, so the trace keeps collective-gated waits off
# the bulk streams:
#   sync:   the 16 bulk load DMAs, final output DMA
#   scalar: x/W/bias loads, h PSUM->SBUF copies, sqrt, outT copies
#   gpsimd: warm-up AG, AllGather bounce DMAs + doorbells
#   vector: bf16 casts, deg copies, rsqrt/Newton, H' scaling, out scale+bias

import numpy as np

N = 8192
D = 128
NCORES = 8
P = 128


def _params(n, ncores):
    nb = n // P  # j-blocks
    rpc = n // ncores  # rows per core
    nhalf = min(512, rpc)  # out slice width (PSUM bank limit)
    nslice = rpc // nhalf
    NCH = nslice  # i-chunks == out slices
    ich = rpc // NCH
    return nb, rpc, nhalf, nslice, NCH, ich


def _build(n=N, d=D, ncores=NCORES):
    from contextlib import ExitStack

    import concourse.bacc as bacc
    import concourse.masks as masks
    import concourse.mybir as mybir
    import concourse.tile as tile

    f32 = mybir.dt.float32
    bf16 = mybir.dt.bfloat16
    mult = mybir.AluOpType.mult
    add = mybir.AluOpType.add

    nb, rpc, nhalf, nslice, NCH, ich = _params(n, ncores)
    lb = rpc // P
    cw = nb * ich
    bpd = min(8, nb)  # j-blocks per load DMA (2MB fp32)
    lbc = lb // NCH
    ngr = nb // bpd  # load groups per chunk

    def chunk_of_block(b):
        return (b % lb) // lbc

    def col_in_chunk(b):
        return lbc * (b // lb) + (b % lb) - chunk_of_block(b) * lbc

    nc = bacc.Bacc("TRN2", target_bir_lowering=False, debug=False, num_devices=ncores)

    adjp = nc.dram_tensor("adjp", [P, NCH * cw], f32, kind="ExternalInput")
    xT = nc.dram_tensor("xT", [d, n], f32, kind="ExternalInput")
    w = nc.dram_tensor("w", [d, d], f32, kind="ExternalInput")
    bias = nc.dram_tensor("bias", [d], f32, kind="ExternalInput")
    out = nc.dram_tensor("out", [rpc, d], f32, kind="ExternalOutput")

    with tile.TileContext(nc) as tc, ExitStack() as ctx:
        singles = ctx.enter_context(tc.tile_pool(name="singles", bufs=1))
        dram = ctx.enter_context(tc.tile_pool(name="dram", bufs=1, space="DRAM"))
        atp = ctx.enter_context(tc.tile_pool(name="atp", bufs=1))
        stp = ctx.enter_context(tc.tile_pool(name="stp", bufs=2))
        xcp = ctx.enter_context(tc.tile_pool(name="xcp", bufs=3))
        psdeg = ctx.enter_context(tc.tile_pool(name="psdeg", bufs=1, space="PSUM"))
        psh = ctx.enter_context(tc.tile_pool(name="psh", bufs=2, space="PSUM"))
        psout = ctx.enter_context(tc.tile_pool(name="psout", bufs=1, space="PSUM"))
        psmisc = ctx.enter_context(tc.tile_pool(name="psmisc", bufs=2, space="PSUM"))

        # ---- warm-up AllGather ----
        wa_in = dram.tile([P], f32, name="wa_in")
        wa_out = dram.tile([ncores * P], f32, name="wa_out", addr_space="Shared")
        wa_sb = singles.tile([1, P], f32)
        nc.gpsimd.memset(wa_sb[:], 0.0)
        nc.gpsimd.dma_start(wa_in[:], wa_sb[:1, :])
        nc.gpsimd.collective_compute(
            "AllGather",
            mybir.AluOpType.bypass,
            replica_groups=[list(range(ncores))],
            ins=[wa_in.opt()],
            outs=[wa_out.opt()],
        )

        # ---- constants ----
        ident = singles.tile([P, P], f32)
        masks.make_identity(nc, ident[:])
        ones_bf = singles.tile([P, 1], bf16)
        nc.gpsimd.memset(ones_bf[:], 1.0)
        ones_row = singles.tile([1, P], f32)
        nc.gpsimd.memset(ones_row[:], 1.0)
        w_sb = singles.tile([d, d], f32)
        nc.scalar.dma_start(w_sb[:], w[:, :])
        bias_row = singles.tile([1, d], f32)
        nc.scalar.dma_start(bias_row[:], bias[:])
        bias_mat = singles.tile([P, d], f32)
        bm_ps = psmisc.tile([P, d], f32, tag="misc")
        nc.tensor.matmul(bm_ps[:], ones_row[:], bias_row[:])
        nc.vector.tensor_copy(bias_mat[:], bm_ps[:])

        # ---- big SBUF residents ----
        AT = atp.tile([P, NCH * cw], bf16)
        Hb = singles.tile([P, nb * d], bf16)  # h then H' in place

        # ---- h = x @ W (dedicated early phase; overlaps the load) ----
        xch = min(1024, n)
        for c0 in range(0, n, xch):
            xc = xcp.tile([d, xch], f32)
            nc.scalar.dma_start(xc[:], xT[:, c0 : c0 + xch])
            for bb in range(xch // P):
                b = c0 // P + bb
                h_ps = psh.tile([P, d], f32)
                nc.tensor.matmul(h_ps[:], xc[:, bb * P : (bb + 1) * P], w_sb[:])
                nc.scalar.copy(Hb[:, b * d : (b + 1) * d], h_ps[:])

        deg_ps = [psdeg.tile([1, ich], f32, name=f"deg_ps{ic}") for ic in range(NCH)]
        out_ps = [psout.tile([P, nhalf], f32, name=f"out_ps{s}") for s in range(nslice)]
        deg_sb = singles.tile([1, rpc], f32)
        dinv_ch = [singles.tile([P, nb // NCH], f32, name=f"dinv_ch{ic}") for ic in range(NCH)]
        ag_outs = []

        def rsqrt_newton(dst, deg_psum, width, tag):
            dgc = singles.tile([P, width], f32, name=f"dgc_{tag}")
            nc.vector.tensor_copy(dgc[:], deg_psum[:])
            sq = singles.tile([P, width], f32, name=f"sq_{tag}")
            nc.scalar.sqrt(sq[:], deg_psum[:])
            r0 = singles.tile([P, width], f32, name=f"r0_{tag}")
            nc.vector.reciprocal(r0[:], sq[:])
            t0 = singles.tile([P, width], f32, name=f"t0_{tag}")
            nc.vector.tensor_mul(t0[:], r0[:], r0[:])
            nc.vector.tensor_mul(t0[:], t0[:], dgc[:])
            nc.vector.tensor_scalar(t0[:], t0[:], -0.5, 1.5, mult, add)
            nc.vector.tensor_mul(dst[:], t0[:], r0[:])

        def ag_chain(ic):
            nc.vector.tensor_copy(deg_sb[:, ic * ich : (ic + 1) * ich], deg_ps[ic][:])
            ag_in = dram.tile([ich], f32, name=f"ag_in{ic}")
            ag_out = dram.tile([ncores * ich], f32, name=f"ag_out{ic}", addr_space="Shared")
            nc.gpsimd.dma_start(ag_in[:], deg_sb[:1, ic * ich : (ic + 1) * ich])
            nc.gpsimd.collective_compute(
                "AllGather",
                mybir.AluOpType.bypass,
                replica_groups=[list(range(ncores))],
                ins=[ag_in.opt()],
                outs=[ag_out.opt()],
            )
            ag_outs.append(ag_out)

        def dinv_chain(ic):
            nbc = nb // NCH
            degc = singles.tile([nbc, P], f32, name=f"degc{ic}")
            nc.gpsimd.dma_start(degc[:], ag_outs[ic][:])
            dgt_ps = psmisc.tile([P, nbc], f32, tag="misc")
            nc.tensor.transpose(dgt_ps[:], degc[:], ident[:nbc, :nbc])
            rsqrt_newton(dinv_ch[ic], dgt_ps, nbc, f"g{ic}")

        def spmm_block(b):
            col = col_in_chunk(b)
            nc.vector.tensor_scalar(
                Hb[:, b * d : (b + 1) * d],
                Hb[:, b * d : (b + 1) * d],
                dinv_ch[chunk_of_block(b)][:, col : col + 1],
                None,
                mult,
            )
            for s in range(nslice):
                nc.tensor.matmul(
                    out_ps[s][:],
                    Hb[:, b * d : (b + 1) * d],
                    AT[:, s * cw + b * ich : s * cw + (b + 1) * ich],
                    start=(b == 0),
                    stop=(b == nb - 1),
                    skip_group_check=True,
                )

        def load_group(ic, g):
            b0 = g * bpd
            lo = ic * cw + b0 * ich
            hi = ic * cw + (b0 + bpd) * ich
            stage = stp.tile([P, bpd * ich], f32)
            nc.sync.dma_start(stage[:], adjp[:, lo:hi])
            nc.vector.tensor_scalar(AT[:, lo:hi], stage[:], 1.0, None, mult)
            for b in range(b0, b0 + bpd):
                nc.tensor.matmul(
                    deg_ps[ic][:],
                    ones_bf[:],
                    AT[:, ic * cw + b * ich : ic * cw + (b + 1) * ich],
                    start=(b == 0),
                    stop=(b == nb - 1),
                )

        # ---- chunk 0 load ----
        for g in range(ngr):
            load_group(0, g)
        if NCH == 1:
            ag_chain(0)
            dinv_chain(0)
            for b in range(nb):
                spmm_block(b)
        else:
            # chunk 1 load with AG#0 + earlier-chunk SpMM woven in at points
            # where their dependencies are already satisfied
            for g in range(ngr):
                load_group(1, g)
                if g == 0:
                    ag_chain(0)  # deg#0 copy lands after cast(c1 g0) in DVE order
                if g == 1:
                    dinv_chain(0)  # AG#0 done by now (warm collective)
                if g >= 2:
                    for b in range((g - 2) * bpd, (g - 2 + 1) * bpd):
                        if chunk_of_block(b) == 0:
                            spmm_block(b)
            ag_chain(1)
            for g in range(ngr - 2, ngr):
                for b in range(g * bpd, (g + 1) * bpd):
                    if chunk_of_block(b) == 0:
                        spmm_block(b)
            dinv_chain(1)
            for b in range(nb):
                if chunk_of_block(b) == 1:
                    spmm_block(b)

        # local dinv for this core's output rows, [p, r] layout
        dloc_ps = psmisc.tile([P, lb], f32, tag="misc")
        for r in range(lb):
            nc.tensor.transpose(
                dloc_ps[:, r : r + 1], deg_sb[:1, r * P : (r + 1) * P], ident[:1, :1]
            )
        dinvl = singles.tile([P, lb], f32)
        rsqrt_newton(dinvl, dloc_ps, lb, "l")

        # ---- finalize ----
        outT_sb = singles.tile([P, rpc], f32)
        for s in range(nslice):
            nc.scalar.copy(outT_sb[:, s * nhalf : (s + 1) * nhalf], out_ps[s][:])
        out_sb = singles.tile([P, lb * d], f32)
        for r in range(lb):
            ob_ps = psmisc.tile([P, d], f32, tag="misc")
            nc.tensor.transpose(ob_ps[:], outT_sb[:, r * P : (r + 1) * P], ident[:])
            nc.vector.tensor_scalar(
                out_sb[:, r * d : (r + 1) * d], ob_ps[:], dinvl[:, r : r + 1], None, mult
            )
            nc.vector.tensor_add(
                out_sb[:, r * d : (r + 1) * d], out_sb[:, r * d : (r + 1) * d], bias_mat[:]
            )
        nc.sync.dma_start(
            out.ap().rearrange("(r p) d -> p r d", p=P),
            out_sb[:].rearrange("p (r d) -> p r d", d=d),
        )

    nc.compile()
    return nc


_NC_CACHE = {}


def _get_nc(n=N, d=D, ncores=NCORES):
    key = (n, d, ncores)
    if key not in _NC_CACHE:
        _NC_CACHE[key] = _build(n, d, ncores)
    return _NC_CACHE[key]


def _pack_shard(adj, c, n, ncores):
    # pack[p, ic, b, i] = adj[c*rpc + ic*ich + i, b*P + p], flattened 2D.
    nb, rpc, nhalf, nslice, NCH, ich = _params(n, ncores)
    shard = adj[c * rpc : (c + 1) * rpc, :]  # [rpc, n]
    t = shard.T.reshape(nb, P, NCH, ich)  # [b, p, ic, i]
    return np.ascontiguousarray(t.transpose(1, 2, 0, 3).reshape(P, NCH * nb * ich))


def run(x, adj, weight, bias, n=N, d=D, ncores=NCORES, trace=False):
    from concourse import bass_utils

    x = np.ascontiguousarray(np.asarray(x, dtype=np.float32))
    adj = np.ascontiguousarray(np.asarray(adj, dtype=np.float32))
    weight = np.ascontiguousarray(np.asarray(weight, dtype=np.float32))
    bias = np.ascontiguousarray(np.asarray(bias, dtype=np.float32))

    xTa = np.ascontiguousarray(x.T)
    in_maps = []
    for c in range(ncores):
        in_maps.append(
            {"adjp": _pack_shard(adj, c, n, ncores), "xT": xTa, "w": weight, "bias": bias}
        )

    nc = _get_nc(n, d, ncores)
    res = bass_utils.run_bass_kernel_spmd(
        nc, in_maps, core_ids=list(range(ncores)), trace=trace
    )
    out = np.concatenate([r["out"] for r in res.results], axis=0)
    return out, res


def kernel(x, adj, weight, bias):
    out, _ = run(x, adj, weight, bias)
    return out


# revision 20
# speedup vs baseline: 1.1186x; 1.0826x over previous
# GCNConv (dense adjacency, symmetric normalization) on 8 trn2 NeuronCores.
#
#   out = D^{-1/2} A D^{-1/2} (x @ W) + bias,   deg = A.sum(axis=1)
#
# Strategy (row-shard, 1D graph partition):
#   - core c owns output rows [1024c, 1024(c+1)). Its 32MB shard of A is
#     laid out host-side in the exact SBUF target layout ("packed"):
#     pack[p, ic, b, i] = A[rows_c[ic*512+i], b*128+p]  - the shard
#     transposed (contraction index j = b*128+p on the partition axis) and
#     i-chunked, so every load DMA is a plain 2D slice with 16KB contiguous
#     per partition (full HBM streaming rate).
#   - The shard streams in fp32 (HWDGE), is cast to bf16 on the vector
#     engine, and stays SBUF-resident (16MB): one HBM pass over A.
#   - deg (row sums of A) = ones^T @ adjT on the tensor engine; a tiny
#     AllGather distributes deg. The local i axis is split in two chunks so
#     AG#0 and the SpMM it unlocks overlap the second half of the load; a
#     dummy warm-up AllGather at t=0 absorbs the ~50us first-collective
#     cost so the real AGs run at ~8us.
#   - dinv = 1/sqrt(deg) via ACT sqrt + DVE reciprocal + one Newton step.
#   - h = x @ W from host-transposed xT (replicated); H' = dinv*h in bf16 is
#     the stationary operand of the SpMM:
#        outT[d, i] += sum_j H'[j, d] * adjT[j, i]
#     accumulated in PSUM over j-blocks, transposed back, scaled by local
#     dinv rows, bias added, DMA'd out.
#
# Engine queues are in-order, so the trace keeps collective-gated waits off
# the bulk streams:
#   sync:   the 16 bulk load DMAs, final output DMA
#   scalar: x/W/bias loads, h PSUM->SBUF copies, sqrt, outT copies
#   gpsimd: warm-up AG, AllGather bounce DMAs + doorbells
#   vector: bf16 casts, deg copies, rsqrt/Newton, H' scaling, out scale+bias

import numpy as np

N = 8192
D = 128
NCORES = 8
P = 128


def _params(n, ncores):
    nb = n // P  # j-blocks
    rpc = n // ncores  # rows per core
    nhalf = min(512, rpc)  # out slice width (PSUM bank limit)
    nslice = rpc // nhalf
    NCH = nslice  # i-chunks == out slices
    ich = rpc // NCH
    return nb, rpc, nhalf, nslice, NCH, ich


def _build(n=N, d=D, ncores=NCORES):
    from contextlib import ExitStack

    import concourse.bacc as bacc
    import concourse.masks as masks
    import concourse.mybir as mybir
    import concourse.tile as tile

    f32 = mybir.dt.float32
    bf16 = mybir.dt.bfloat16
    mult = mybir.AluOpType.mult
    add = mybir.AluOpType.add

    nb, rpc, nhalf, nslice, NCH, ich = _params(n, ncores)
    lb = rpc // P
    cw = nb * ich
    bpd = min(8, nb)  # j-blocks per load DMA (2MB fp32)
    lbc = lb // NCH
    ngr = nb // bpd  # load groups per chunk

    def chunk_of_block(b):
        return (b % lb) // lbc

    def col_in_chunk(b):
        return lbc * (b // lb) + (b % lb) - chunk_of_block(b) * lbc

    nc = bacc.Bacc("TRN2", target_bir_lowering=False, debug=False, num_devices=ncores)

    adjp = nc.dram_tensor("adjp", [P, NCH * cw], f32, kind="ExternalInput")
    xT = nc.dram_tensor("xT", [d, n], f32, kind="ExternalInput")
    w = nc.dram_tensor("w", [d, d], f32, kind="ExternalInput")
    bias = nc.dram_tensor("bias", [d], f32, kind="ExternalInput")
    out = nc.dram_tensor("out", [rpc, d], f32, kind="ExternalOutput")

    with tile.TileContext(nc) as tc, ExitStack() as ctx:
        singles = ctx.enter_context(tc.tile_pool(name="singles", bufs=1))
        dram = ctx.enter_context(tc.tile_pool(name="dram", bufs=1, space="DRAM"))
        atp = ctx.enter_context(tc.tile_pool(name="atp", bufs=1))
        stp = ctx.enter_context(tc.tile_pool(name="stp", bufs=2))
        xcp = ctx.enter_context(tc.tile_pool(name="xcp", bufs=3))
        psdeg = ctx.enter_context(tc.tile_pool(name="psdeg", bufs=1, space="PSUM"))
        psh = ctx.enter_context(tc.tile_pool(name="psh", bufs=2, space="PSUM"))
        psout = ctx.enter_context(tc.tile_pool(name="psout", bufs=1, space="PSUM"))
        psmisc = ctx.enter_context(tc.tile_pool(name="psmisc", bufs=2, space="PSUM"))

        # ---- warm-up AllGather ----
        wa_in = dram.tile([P], f32, name="wa_in")
        wa_out = dram.tile([ncores * P], f32, name="wa_out", addr_space="Shared")
        wa_sb = singles.tile([1, P], f32)
        nc.gpsimd.memset(wa_sb[:], 0.0)
        nc.gpsimd.dma_start(wa_in[:], wa_sb[:1, :])
        nc.gpsimd.collective_compute(
            "AllGather",
            mybir.AluOpType.bypass,
            replica_groups=[list(range(ncores))],
            ins=[wa_in.opt()],
            outs=[wa_out.opt()],
        )

        # ---- constants ----
        ident = singles.tile([P, P], f32)
        masks.make_identity(nc, ident[:])
        ones_bf = singles.tile([P, 1], bf16)
        nc.gpsimd.memset(ones_bf[:], 1.0)
        ones_row = singles.tile([1, P], f32)
        nc.gpsimd.memset(ones_row[:], 1.0)
        w_sb = singles.tile([d, d], f32)
        nc.scalar.dma_start(w_sb[:], w[:, :])
        bias_row = singles.tile([1, d], f32)
        nc.scalar.dma_start(bias_row[:], bias[:])
        bias_mat = singles.tile([P, d], f32)
        bm_ps = psmisc.tile([P, d], f32, tag="misc")
        nc.tensor.matmul(bm_ps[:], ones_row[:], bias_row[:])
        nc.vector.tensor_copy(bias_mat[:], bm_ps[:])

        # ---- big SBUF residents ----
        AT = atp.tile([P, NCH * cw], bf16)
        Hb = singles.tile([P, nb * d], bf16)  # h then H' in place

        # ---- h = x @ W (traced after chunk-0's deg so PE prioritizes deg) --
        xch = min(1024, n)

        def h_all():
            for c0 in range(0, n, xch):
                xc = xcp.tile([d, xch], f32)
                nc.scalar.dma_start(xc[:], xT[:, c0 : c0 + xch])
                for bb in range(xch // P):
                    b = c0 // P + bb
                    h_ps = psh.tile([P, d], f32)
                    nc.tensor.matmul(h_ps[:], xc[:, bb * P : (bb + 1) * P], w_sb[:])
                    nc.scalar.copy(Hb[:, b * d : (b + 1) * d], h_ps[:])

        deg_ps = [psdeg.tile([1, ich], f32, name=f"deg_ps{ic}") for ic in range(NCH)]
        out_ps = [psout.tile([P, nhalf], f32, name=f"out_ps{s}") for s in range(nslice)]
        deg_sb = singles.tile([1, rpc], f32)
        dinv_ch = [singles.tile([P, nb // NCH], f32, name=f"dinv_ch{ic}") for ic in range(NCH)]
        ag_outs = []

        def rsqrt_newton(dst, deg_psum, width, tag):
            dgc = singles.tile([P, width], f32, name=f"dgc_{tag}")
            nc.vector.tensor_copy(dgc[:], deg_psum[:])
            sq = singles.tile([P, width], f32, name=f"sq_{tag}")
            nc.scalar.sqrt(sq[:], deg_psum[:])
            r0 = singles.tile([P, width], f32, name=f"r0_{tag}")
            nc.vector.reciprocal(r0[:], sq[:])
            t0 = singles.tile([P, width], f32, name=f"t0_{tag}")
            nc.vector.tensor_mul(t0[:], r0[:], r0[:])
            nc.vector.tensor_mul(t0[:], t0[:], dgc[:])
            nc.vector.tensor_scalar(t0[:], t0[:], -0.5, 1.5, mult, add)
            nc.vector.tensor_mul(dst[:], t0[:], r0[:])

        def ag_chain(ic):
            nc.vector.tensor_copy(deg_sb[:, ic * ich : (ic + 1) * ich], deg_ps[ic][:])
            ag_in = dram.tile([ich], f32, name=f"ag_in{ic}")
            ag_out = dram.tile([ncores * ich], f32, name=f"ag_out{ic}", addr_space="Shared")
            nc.gpsimd.dma_start(ag_in[:], deg_sb[:1, ic * ich : (ic + 1) * ich])
            nc.gpsimd.collective_compute(
                "AllGather",
                mybir.AluOpType.bypass,
                replica_groups=[list(range(ncores))],
                ins=[ag_in.opt()],
                outs=[ag_out.opt()],
            )
            ag_outs.append(ag_out)

        def dinv_chain(ic):
            nbc = nb // NCH
            degc = singles.tile([nbc, P], f32, name=f"degc{ic}")
            nc.gpsimd.dma_start(degc[:], ag_outs[ic][:])
            dgt_ps = psmisc.tile([P, nbc], f32, tag="misc")
            nc.tensor.transpose(dgt_ps[:], degc[:], ident[:nbc, :nbc])
            rsqrt_newton(dinv_ch[ic], dgt_ps, nbc, f"g{ic}")

        def spmm_block(b):
            col = col_in_chunk(b)
            nc.vector.tensor_scalar(
                Hb[:, b * d : (b + 1) * d],
                Hb[:, b * d : (b + 1) * d],
                dinv_ch[chunk_of_block(b)][:, col : col + 1],
                None,
                mult,
            )
            for s in range(nslice):
                nc.tensor.matmul(
                    out_ps[s][:],
                    Hb[:, b * d : (b + 1) * d],
                    AT[:, s * cw + b * ich : s * cw + (b + 1) * ich],
                    start=(b == 0),
                    stop=(b == nb - 1),
                    skip_group_check=True,
                )

        def load_group(ic, g):
            b0 = g * bpd
            lo = ic * cw + b0 * ich
            hi = ic * cw + (b0 + bpd) * ich
            stage = stp.tile([P, bpd * ich], f32)
            nc.sync.dma_start(stage[:], adjp[:, lo:hi])
            nc.vector.tensor_scalar(AT[:, lo:hi], stage[:], 1.0, None, mult)
            for b in range(b0, b0 + bpd):
                nc.tensor.matmul(
                    deg_ps[ic][:],
                    ones_bf[:],
                    AT[:, ic * cw + b * ich : ic * cw + (b + 1) * ich],
                    start=(b == 0),
                    stop=(b == nb - 1),
                )

        # ---- schedule: nothing collective-gated ever sits ahead of the load
        # in any engine queue. AG#0 gets a ~50us hidden window; SpMM chunk 0
        # runs right after the load, filling AG#1's window. ----
        for g in range(ngr):
            load_group(0, g)
        h_all()
        ag_chain(0)
        for ic in range(1, NCH):
            for g in range(ngr):
                load_group(ic, g)
            ag_chain(ic)
        dinv_chain(0)
        for b in range(nb):
            if chunk_of_block(b) == 0:
                spmm_block(b)
        for ic in range(1, NCH):
            dinv_chain(ic)
            for b in range(nb):
                if chunk_of_block(b) == ic:
                    spmm_block(b)

        # local dinv for this core's output rows, [p, r] layout
        dloc_ps = psmisc.tile([P, lb], f32, tag="misc")
        for r in range(lb):
            nc.tensor.transpose(
                dloc_ps[:, r : r + 1], deg_sb[:1, r * P : (r + 1) * P], ident[:1, :1]
            )
        dinvl = singles.tile([P, lb], f32)
        rsqrt_newton(dinvl, dloc_ps, lb, "l")

        # ---- finalize ----
        outT_sb = singles.tile([P, rpc], f32)
        for s in range(nslice):
            nc.scalar.copy(outT_sb[:, s * nhalf : (s + 1) * nhalf], out_ps[s][:])
        out_sb = singles.tile([P, lb * d], f32)
        for r in range(lb):
            ob_ps = psmisc.tile([P, d], f32, tag="misc")
            nc.tensor.transpose(ob_ps[:], outT_sb[:, r * P : (r + 1) * P], ident[:])
            nc.vector.tensor_scalar(
                out_sb[:, r * d : (r + 1) * d], ob_ps[:], dinvl[:, r : r + 1], None, mult
            )
            nc.vector.tensor_add(
                out_sb[:, r * d : (r + 1) * d], out_sb[:, r * d : (r + 1) * d], bias_mat[:]
            )
        nc.sync.dma_start(
            out.ap().rearrange("(r p) d -> p r d", p=P),
            out_sb[:].rearrange("p (r d) -> p r d", d=d),
        )

    nc.compile()
    return nc


_NC_CACHE = {}


def _get_nc(n=N, d=D, ncores=NCORES):
    key = (n, d, ncores)
    if key not in _NC_CACHE:
        _NC_CACHE[key] = _build(n, d, ncores)
    return _NC_CACHE[key]


def _pack_shard(adj, c, n, ncores):
    # pack[p, ic, b, i] = adj[c*rpc + ic*ich + i, b*P + p], flattened 2D.
    nb, rpc, nhalf, nslice, NCH, ich = _params(n, ncores)
    shard = adj[c * rpc : (c + 1) * rpc, :]  # [rpc, n]
    t = shard.T.reshape(nb, P, NCH, ich)  # [b, p, ic, i]
    return np.ascontiguousarray(t.transpose(1, 2, 0, 3).reshape(P, NCH * nb * ich))


def run(x, adj, weight, bias, n=N, d=D, ncores=NCORES, trace=False):
    from concourse import bass_utils

    x = np.ascontiguousarray(np.asarray(x, dtype=np.float32))
    adj = np.ascontiguousarray(np.asarray(adj, dtype=np.float32))
    weight = np.ascontiguousarray(np.asarray(weight, dtype=np.float32))
    bias = np.ascontiguousarray(np.asarray(bias, dtype=np.float32))

    xTa = np.ascontiguousarray(x.T)
    in_maps = []
    for c in range(ncores):
        in_maps.append(
            {"adjp": _pack_shard(adj, c, n, ncores), "xT": xTa, "w": weight, "bias": bias}
        )

    nc = _get_nc(n, d, ncores)
    res = bass_utils.run_bass_kernel_spmd(
        nc, in_maps, core_ids=list(range(ncores)), trace=trace
    )
    out = np.concatenate([r["out"] for r in res.results], axis=0)
    return out, res


def kernel(x, adj, weight, bias):
    out, _ = run(x, adj, weight, bias)
    return out
